# revision 1
# baseline (speedup 1.0000x reference)
"""Trainium2 Bass kernel for nn_DirectedEdgeEncoder (directed-GCN edge encoder).

Self-contained: hardcodes problem shapes (N=50000, E=800000, IN=128, HID=64,
OUT=32, 8 cores) and builds/runs an SPMD Bass program on NeuronCores 0-7.

Math (reference refactored):
  With self-loops appended, per-edge norm = in_deg[dst]^-.5 * out_deg[src]^-.5
  is IDENTICAL for the forward and flipped conv directions (alpha=beta=0.5).
  Let A[t,s] = sum of norms of edges s->t (incl. self loops). Then
    s_mu||s_logstd = A.T @ ( relu(A @ (s@W1s.T + b1s)) @ W2s.T + b2s )
    t_mu||t_logstd = A   @ ( relu(A.T @ (t@W1t.T + b1t)) @ W2t.T + b2t )
  where W1s = [sm1_W; sl1_W] (128x128), W2s = blockdiag(sm2_W, sl2_W) (64x128).
  outputs: s_out = s_mu + noise_s*exp(s_logstd)/5  (same for t).

  norm factorizes: norm_e = a[dst_e]*b[src_e], a=in_deg^-.5, b=out_deg^-.5.
  The gather-side factor is folded into the stored node tables
  (h1s' = b*h1s for forward gathers, h1t' = a*h1t for backward gathers, etc);
  the scatter-side factor rides in the per-edge selection matrices.
  Biases commute through the conv as  A @ (h + 1 b^T) = A@h + rowsum(A) (x) b,
  implemented as a rank-1 (K=1) matmul seeding the PSUM accumulation.

Device mapping per core (nodes sharded 6400/core, NPAD=51200):
  dense1 (own nodes) -> AllGather h1' -> sparse pass1 (fwd for s, bwd for t):
  per 256-node supertile, one indirect-DMA gather of all edge messages,
  per 128-edge block one DVE tensor_scalar builds S[e,n]=sc_e*(iota==col_e),
  PE accumulates z1T[:,n] += msgs.T @ S in PSUM (float32r, moving dim 256);
  relu (ACT) -> dense2 -> AllGather g' -> sparse pass2 (bwd for s, fwd for t)
  -> combine with noise (ACT exp + DVE) -> transposed outputs [32, 6400].
"""

import math
import numpy as np

import concourse.bass as bass
import concourse.mybir as mybir
import concourse.tile as tile
from concourse import bacc
from concourse import bass_utils

F32 = mybir.dt.float32
F32R = mybir.dt.float32r
I32 = mybir.dt.int32
I16 = mybir.dt.int16
BF16 = mybir.dt.bfloat16
AF = mybir.ActivationFunctionType
OP = mybir.AluOpType

# problem constants
N, E, IN, HID, OUT = 50000, 800000, 128, 64, 32
F1 = 2 * HID   # 128, concat hidden width
F2 = 2 * OUT   # 64, concat output width
LN5INV = float(np.log(np.float32(1.0) / np.float32(5.0)))

# sharding constants
C = 8
SUP = 256
NLOC = 6400          # nodes per core; 25 supertiles
NPAD = C * NLOC      # 51200
NSUP = NLOC // SUP   # 25
TIL = 128


# ----------------------------------------------------------------- host prep

def _pad_rows(x, rows):
    out = np.zeros((rows, x.shape[1]), np.float32)
    out[: x.shape[0]] = x
    return out


def _build_direction(scatter, gather, sc_vals, npad, nloc, sup, ncores, nsup):
    """Sort edges by (scatter supertile, gather-table half); lay out per
    (core, supertile, half) as ranks -> slot (partition rank%128, block
    rank//128, second half offset by KA blocks).  Returns per-half block
    counts [KA, KB] and per-core (gatheridx, col, sc) slot arrays.
    gatheridx values are LOCAL to their table half (for int16 dma_gather)."""
    half_rows = npad // 2
    gsup = scatter // sup                      # global supertile id
    half = gather // half_rows                 # which gather-table half
    key = gsup * 2 + half
    order = np.argsort(key, kind="stable")
    key_s = key[order]
    nkeys = (npad // sup) * 2
    cnt = np.bincount(key_s, minlength=nkeys)
    ka = int(math.ceil(cnt[0::2].max() / 128.0))
    kb = int(math.ceil(cnt[1::2].max() / 128.0))
    starts = np.zeros(nkeys + 1, np.int64)
    np.cumsum(cnt, out=starts[1:])
    rank = np.arange(len(order)) - starts[key_s]
    p = (rank % 128).astype(np.int64)
    blk = (rank // 128).astype(np.int64) + (key_s % 2) * ka
    sup_per_core = nloc // sup
    gsup_s = key_s // 2
    core = gsup_s // sup_per_core
    supl = gsup_s % sup_per_core

    kblk = ka + kb
    src = np.zeros((ncores, nsup, 128, kblk), np.int32)     # pad -> row 0
    col = np.full((ncores, nsup, 128, kblk), -1.0, np.float32)
    sc = np.zeros((ncores, nsup, 128, kblk), np.float32)
    src[core, supl, p, blk] = (gather[order] % half_rows).astype(np.int32)
    col[core, supl, p, blk] = (scatter[order] - gsup_s * sup).astype(np.float32)
    sc[core, supl, p, blk] = sc_vals[order].astype(np.float32)
    return [ka, kb], src, col, sc


def _repad_halves(src, col, sc, kh_from, kh_to):
    """Re-pad per-half block counts [ka,kb] -> uniform [ka',kb']."""
    if kh_from == kh_to:
        return src, col, sc
    outs = []
    for arr, fill in ((src, 0), (col, -1.0), (sc, 0.0)):
        out = np.full(arr.shape[:-1] + (sum(kh_to),), fill, arr.dtype)
        out[..., :kh_from[0]] = arr[..., :kh_from[0]]
        out[..., kh_to[0]:kh_to[0] + kh_from[1]] = arr[..., kh_from[0]:]
        outs.append(out)
    return outs


def _wrap_idx16(src, kh):
    """Slot-array gather indices [C, NSUP, 128, KA+KB] (int32, half-local)
    -> dma_gather idx tiles [C, NSUP, 128, (KA+KB)*8] int16: per half,
    index i at [16-wrap partition i%16, col i//16], replicated x8 down."""
    ncores, nsup = src.shape[0], src.shape[1]
    ka, kb = kh
    out = np.zeros((ncores, nsup, 128, (ka + kb) * 8), np.int16)
    for h, (b0, nb) in enumerate(((0, ka), (ka, kb))):
        if nb == 0:
            continue
        # slot rank i (within half) = block*128 + partition
        sl = src[..., b0:b0 + nb]                       # [C,S,128,nb]
        flat = sl.transpose(0, 1, 3, 2).reshape(ncores, nsup, nb * 128)
        w = flat.reshape(ncores, nsup, nb * 8, 16).astype(np.int16)
        w = w.transpose(0, 1, 3, 2)                     # [C,S,16,nb*8]
        out[..., :16, b0 * 8:(b0 + nb) * 8] = w
        for g in range(1, 8):
            out[..., g * 16:(g + 1) * 16, b0 * 8:(b0 + nb) * 8] = w
    return out


def prepare_inputs(inputs, ncores=C, nloc=NLOC, sup=SUP, n=N):
    """Shard + preprocess full inputs into per-core in_maps. Pure numpy."""
    npad = ncores * nloc
    nsup = nloc // sup
    f32 = np.float32

    s = np.asarray(inputs["s"], f32)
    t = np.asarray(inputs["t"], f32)
    ei = np.asarray(inputs["edge_index"], np.int64)
    noise_s = np.asarray(inputs["noise_s"], f32)
    noise_t = np.asarray(inputs["noise_t"], f32)

    W1s = np.vstack([inputs["sm1_W"], inputs["sl1_W"]]).astype(f32)   # [F1, IN]
    W1t = np.vstack([inputs["tm1_W"], inputs["tl1_W"]]).astype(f32)
    b1s = np.concatenate([inputs["sm1_b"], inputs["sl1_b"]]).astype(f32)
    b1t = np.concatenate([inputs["tm1_b"], inputs["tl1_b"]]).astype(f32)
    f2, f1 = 2 * inputs["sm2_W"].shape[0], 2 * inputs["sm1_W"].shape[0]
    W2s = np.zeros((f2, f1), f32)
    W2s[: f2 // 2, : f1 // 2] = inputs["sm2_W"]
    W2s[f2 // 2:, f1 // 2:] = inputs["sl2_W"]
    W2t = np.zeros((f2, f1), f32)
    W2t[: f2 // 2, : f1 // 2] = inputs["tm2_W"]
    W2t[f2 // 2:, f1 // 2:] = inputs["tl2_W"]
    b2s = np.concatenate([inputs["sm2_b"], inputs["sl2_b"]]).astype(f32)
    b2t = np.concatenate([inputs["tm2_b"], inputs["tl2_b"]]).astype(f32)

    loops = np.arange(n, dtype=np.int64)
    src = np.concatenate([ei[0], loops])
    dst = np.concatenate([ei[1], loops])
    deg_in = np.bincount(dst, minlength=npad).astype(f32)
    deg_out = np.bincount(src, minlength=npad).astype(f32)
    a = (np.maximum(deg_in, 1.0) ** -0.5).astype(f32)    # scatter-side fwd
    b = (np.maximum(deg_out, 1.0) ** -0.5).astype(f32)
    rs_fwd = a * np.bincount(dst, weights=b[src].astype(np.float64),
                             minlength=npad).astype(f32)
    rs_bwd = b * np.bincount(src, weights=a[dst].astype(np.float64),
                             minlength=npad).astype(f32)

    # forward direction (matrix A): scatter by dst, gather at src, factor a[dst]
    khf, srcF, colF, scF = _build_direction(dst, src, a[dst], npad, nloc, sup,
                                            ncores, nsup)
    # backward direction (A.T): scatter by src, gather at dst, factor b[src]
    khb, srcB, colB, scB = _build_direction(src, dst, b[src], npad, nloc, sup,
                                            ncores, nsup)
    kh = [max(khf[0], khb[0]), max(khf[1], khb[1])]
    srcF, colF, scF = _repad_halves(srcF, colF, scF, khf, kh)
    srcB, colB, scB = _repad_halves(srcB, colB, scB, khb, kh)
    kblk = kh[0] + kh[1]
    idxF = _wrap_idx16(srcF, kh)
    idxB = _wrap_idx16(srcB, kh)

    def colsc(col, sc):  # pack [.., 128, 2K]
        return np.concatenate([col, sc], axis=-1)

    sT = _pad_rows(s, npad).T.copy()           # [IN, npad]
    tT = _pad_rows(t, npad).T.copy()
    # /DIVIDER folded into the noise here (out = mu + (noise/5) * exp(logstd))
    nsT = (_pad_rows(noise_s, npad).T / np.float32(5.0)).astype(f32)  # [OUT, npad]
    ntT = (_pad_rows(noise_t, npad).T / np.float32(5.0)).astype(f32)

    iota = np.tile(np.arange(sup, dtype=f32), (128, 1)).copy()

    import ml_dtypes
    bf16 = ml_dtypes.bfloat16
    in_maps = []
    for c in range(ncores):
        r = slice(c * nloc, (c + 1) * nloc)
        in_maps.append({
            "xsT": np.ascontiguousarray(sT[:, r]),
            "xtT": np.ascontiguousarray(tT[:, r]),
            "w1sT": np.ascontiguousarray(W1s.T),
            "w1tT": np.ascontiguousarray(W1t.T),
            "w2sT": np.ascontiguousarray(W2s.T),
            "w2tT": np.ascontiguousarray(W2t.T),
            "b1s": b1s[None, :].astype(bf16),
            "b1t": b1t[None, :].astype(bf16),
            "b2s": b2s[None, :].astype(bf16),
            "b2t": b2t[None, :].astype(bf16),
            "avw": a[r].reshape(nloc // 128, 128).T.copy(),
            "bvw": b[r].reshape(nloc // 128, 128).T.copy(),
            "rsF": rs_fwd[None, r].astype(bf16),
            "rsB": rs_bwd[None, r].astype(bf16),
            "iota": iota,
            "idxF": idxF[c], "colscF": colsc(colF[c], scF[c]),
            "idxB": idxB[c], "colscB": colsc(colB[c], scB[c]),
            "srcF": srcF[c], "srcB": srcB[c],   # debug only (unused by NEFF)
            "nsT": np.ascontiguousarray(nsT[:, r]),
            "ntT": np.ascontiguousarray(ntT[:, r]),
        })
    cfg = dict(ncores=ncores, nloc=nloc, npad=npad, sup=sup, nsup=nsup,
               kblk=kblk, ka=kh[0], kb=kh[1], fin=s.shape[1], f1=W1s.shape[0],
               f2=W2s.shape[0], fo=W2s.shape[0] // 2, n=n)
    return in_maps, cfg


# ------------------------------------------------------------ device program

def build_program(cfg, enable_asserts=False, debug_taps=False, repeat=1, skip_collectives=False, ablate=()):
    ncores = cfg["ncores"]
    nloc, npad, sup, nsup = cfg["nloc"], cfg["npad"], cfg["sup"], cfg["nsup"]
    kblk, fin, f1, f2, fo = (cfg["kblk"], cfg["fin"], cfg["f1"], cfg["f2"],
                             cfg["fo"])
    ntil = nloc // TIL

    nswq = 4
    nc = bacc.Bacc("TRN2", target_bir_lowering=False, debug=False,
                   enable_asserts=enable_asserts, num_devices=ncores,
                   dynamic_dma_scratch_size=65536, num_swdge_queues=nswq)
    qctr = [0]

    def din(name, shape, dtype=F32):
        return nc.dram_tensor(name, shape, dtype, kind="ExternalInput")

    xsT = din("xsT", [fin, nloc], F32R)
    xtT = din("xtT", [fin, nloc], F32R)
    w1sT = din("w1sT", [fin, f1], F32R)
    w1tT = din("w1tT", [fin, f1], F32R)
    w2sT = din("w2sT", [f1, f2], F32R)
    w2tT = din("w2tT", [f1, f2], F32R)
    b1s = din("b1s", [1, f1], BF16)
    b1t = din("b1t", [1, f1], BF16)
    b2s = din("b2s", [1, f2], BF16)
    b2t = din("b2t", [1, f2], BF16)
    avw = din("avw", [128, nloc // TIL])
    bvw = din("bvw", [128, nloc // TIL])
    rsF = din("rsF", [1, nloc], BF16)
    rsB = din("rsB", [1, nloc], BF16)
    iota = din("iota", [128, sup])
    idxF = din("idxF", [nsup, 128, kblk * 8], I16)
    colscF = din("colscF", [nsup, 128, 2 * kblk])
    idxB = din("idxB", [nsup, 128, kblk * 8], I16)
    colscB = din("colscB", [nsup, 128, 2 * kblk])
    nsT = din("nsT", [fo, nloc])
    ntT = din("ntT", [fo, nloc])

    souT = nc.dram_tensor("souT", [fo, nloc], F32, kind="ExternalOutput")
    touT = nc.dram_tensor("touT", [fo, nloc], F32, kind="ExternalOutput")
    dbg_h1s = dbg_gs = dbg_msgs = None
    if debug_taps:
        dbg_h1s = nc.dram_tensor("dbg_h1s", [npad, f1], F32,
                                 kind="ExternalOutput")
        dbg_gs = nc.dram_tensor("dbg_gs", [npad, f2], F32,
                                kind="ExternalOutput")
        dbg_msgs = nc.dram_tensor("dbg_msgs", [128, kblk * f1], F32,
                                  kind="ExternalOutput")

    cc_space = "Shared" if ncores > 4 else "Local"
    h1s_own = nc.dram_tensor("h1s_own", [nloc, f1], BF16, kind="Internal")
    h1t_own = nc.dram_tensor("h1t_own", [nloc, f1], BF16, kind="Internal")
    h1s_p = nc.dram_tensor("h1s_p", [npad, f1], BF16, kind="Internal",
                           addr_space=cc_space)
    h1t_p = nc.dram_tensor("h1t_p", [npad, f1], BF16, kind="Internal",
                           addr_space=cc_space)
    gs_own = nc.dram_tensor("gs_own", [nloc, f2], F32R, kind="Internal")
    gt_own = nc.dram_tensor("gt_own", [nloc, f2], F32R, kind="Internal")
    gs_p = nc.dram_tensor("gs_p", [npad, f2], F32R, kind="Internal",
                          addr_space=cc_space)
    gt_p = nc.dram_tensor("gt_p", [npad, f2], F32R, kind="Internal",
                          addr_space=cc_space)

    groups = [list(range(ncores))]

    with tile.TileContext(nc) as tc:
      if skip_collectives:
          # timing-only variant: gathers must not touch uninitialized HBM
          with tc.tile_pool(name="zinit", bufs=1) as zp:
              zt = zp.tile([128, 3200], F32R, tag="zt")
              nc.gpsimd.memset(zt[:], 0.0)
              for tbl, fdim in ((h1s_p, f1), (h1t_p, f1),
                                (gs_p, f2), (gt_p, f2)):
                  view = tbl[:, :].rearrange("(nn p) f -> p nn f", p=128)
                  zt3 = zt[:].rearrange("p (nn f) -> p nn f", f=fdim)
                  nn_tot, nn_ch = view.shape[1], 3200 // fdim
                  for c0 in range(0, nn_tot, nn_ch):
                      cw = min(nn_ch, nn_tot - c0)
                      nc.gpsimd.dma_start(view[:, c0:c0 + cw, :],
                                          zt3[:, :cw, :])
      for _rep in range(repeat):
        with tc.tile_pool(name="const", bufs=1) as cp:
            w1s_sb = cp.tile([fin, f1], F32R, tag="w1s")
            nc.sync.dma_start(w1s_sb[:], w1sT[:, :])
            w1t_sb = cp.tile([fin, f1], F32R, tag="w1t")
            nc.sync.dma_start(w1t_sb[:], w1tT[:, :])
            w2s_sb = cp.tile([f1, f2], F32R, tag="w2s")
            nc.sync.dma_start(w2s_sb[:], w2sT[:, :])
            w2t_sb = cp.tile([f1, f2], F32R, tag="w2t")
            nc.sync.dma_start(w2t_sb[:], w2tT[:, :])
            b1s_sb = cp.tile([1, f1], BF16, tag="b1s")
            nc.sync.dma_start(b1s_sb[:], b1s[:, :])
            b1t_sb = cp.tile([1, f1], BF16, tag="b1t")
            nc.sync.dma_start(b1t_sb[:], b1t[:, :])
            b2s_sb = cp.tile([1, f2], BF16, tag="b2s")
            nc.sync.dma_start(b2s_sb[:], b2s[:, :])
            b2t_sb = cp.tile([1, f2], BF16, tag="b2t")
            nc.sync.dma_start(b2t_sb[:], b2t[:, :])
            iota_sb = cp.tile([128, sup], F32, tag="iota")
            nc.sync.dma_start(iota_sb[:], iota[:, :])
            avw_sb = cp.tile([128, nloc // TIL], F32, tag="avw")
            nc.sync.dma_start(avw_sb[:], avw[:, :])
            bvw_sb = cp.tile([128, nloc // TIL], F32, tag="bvw")
            nc.sync.dma_start(bvw_sb[:], bvw[:, :])
            rsF_sb = cp.tile([1, nloc], BF16, tag="rsF")
            nc.sync.dma_start(rsF_sb[:], rsF[:, :])
            rsB_sb = cp.tile([1, nloc], BF16, tag="rsB")
            nc.sync.dma_start(rsB_sb[:], rsB[:, :])

            # ---------------- dense layer 1 on own nodes ----------------
            with tc.tile_pool(name="d1", bufs=3) as sb, \
                 tc.tile_pool(name="d1ps", bufs=4, space="PSUM") as ps:
                for j in range(ntil):
                    rows = slice(j * TIL, (j + 1) * TIL)
                    for xT, w1, svw, h1own in (
                        (xsT, w1s_sb, bvw_sb, h1s_own),   # fwd gathers: b*h
                        (xtT, w1t_sb, avw_sb, h1t_own),   # bwd gathers: a*h
                    ):
                        xt = sb.tile([fin, TIL], F32R, tag="xt")
                        nc.sync.dma_start(xt[:], xT[:, rows])
                        p = ps.tile([TIL, f1], F32, tag="d1p")
                        nc.tensor.matmul(p[:], lhsT=xt[:],
                                         rhs=w1[:],
                                         start=True, stop=True)
                        h1sb = sb.tile([TIL, f1], BF16, tag="h1sb")
                        nc.scalar.activation(h1sb[:], p[:], AF.Copy,
                                             scale=svw[:, j:j + 1])
                        nc.sync.dma_start(h1own[rows, :], h1sb[:])

            if not skip_collectives:
                nc.gpsimd.collective_compute(
                    "AllGather", OP.bypass, replica_groups=groups,
                    ins=[h1s_own[:, :].opt()], outs=[h1s_p[:, :].opt()])
                nc.gpsimd.collective_compute(
                    "AllGather", OP.bypass, replica_groups=groups,
                    ins=[h1t_own[:, :].opt()], outs=[h1t_p[:, :].opt()])

            # ---------------- sparse passes ----------------
            halfrows = npad // 2
            kab = [cfg["ka"], cfg["kb"]]

            def sparse_pass(sup_i, *, src_d, colsc_d, rs_d, table, fmsg,
                            bias_sb, zparts, pools, epilogue, msg_tap=None,
                            mdt=F32R):
                sbm, sbg, sbs, psp = pools
                srct = sbm.tile([128, kblk * 8], I16, tag="src")
                nc.sync.dma_start(srct[:], src_d[sup_i])
                cst = sbm.tile([128, 2 * kblk], F32, tag="colsc")
                nc.sync.dma_start(cst[:], colsc_d[sup_i])
                rst = rs_d[0:1, sup_i * sup:(sup_i + 1) * sup]
                msgs = sbg.tile([128, kblk * fmsg], mdt, tag=f"msgs{fmsg}")
                msgs3 = msgs[:].rearrange("p (k f) -> p k f", k=kblk)
                GCH = 8    # dma_gather HW limit: 1024 idxs (64/16-lane) per op
                if "gather" not in ablate:
                    for h, (b0, nb) in enumerate(((0, kab[0]), (kab[0], kab[1]))):
                        for c0 in range(0, nb, GCH):
                            cn = min(GCH, nb - c0)
                            qctr[0] += 1
                            nc.gpsimd.dma_gather(
                                out_ap=msgs3[:, b0 + c0:b0 + c0 + cn, :],
                                in_ap=table[h * halfrows:(h + 1) * halfrows, :],
                                idxs_ap=srct[:, (b0 + c0) * 8:(b0 + c0 + cn) * 8],
                                num_idxs=cn * 128, num_idxs_reg=cn * 128,
                                elem_size=fmsg, queue_num=qctr[0] % nswq)
                if msg_tap is not None:
                    nc.gpsimd.dma_start(msg_tap[:, :], msgs[:])
                if "epi" in ablate:
                    return
                zp = psp.tile([zparts, sup], F32, tag=f"z{zparts}")
                nc.tensor.matmul(zp[:], lhsT=bias_sb[:], rhs=rst,
                                 start=True, stop=("mm" in ablate))
                for k in range(kblk):
                    S = sbs.tile([128, sup], mdt, tag="S")
                    if "sbuild" not in ablate:
                        nc.vector.tensor_scalar(
                            out=S[:], in0=iota_sb[:],
                            scalar1=cst[:, k:k + 1],
                            scalar2=cst[:, kblk + k:kblk + k + 1],
                            op0=OP.is_equal, op1=OP.mult)
                    if "mm" not in ablate:
                        nc.tensor.matmul(
                            zp[:],
                            lhsT=msgs[:, k * fmsg:(k + 1) * fmsg],
                            rhs=S[:],
                            start=False, stop=(k == kblk - 1))
                epilogue(sup_i, zp)

            # pass 1 -> r1 -> dense2 -> g_own
            def make_p1_epilogue(w2_sb, svw, g_own, pools):
                sb2, ps2 = pools

                def epi(sup_i, zp):
                    r1 = sb2.tile([f1, sup], F32R, tag="r1")
                    nc.scalar.activation(r1[:], zp[:], AF.Relu)
                    for h in range(sup // TIL):
                        j = sup_i * (sup // TIL) + h
                        rows = slice(j * TIL, (j + 1) * TIL)
                        gp = ps2.tile([TIL, f2], F32, tag="gp")
                        nc.tensor.matmul(
                            gp[:],
                            lhsT=r1[:, h * TIL:(h + 1) * TIL],
                            rhs=w2_sb[:], start=True, stop=True)
                        gsb = sb2.tile([TIL, f2], F32R, tag="gsb")
                        nc.scalar.activation(gsb[:], gp[:], AF.Copy,
                                             scale=svw[:, j:j + 1])
                        nc.sync.dma_start(g_own[rows, :], gsb[:])
                return epi

            # pass 2 -> combine with noise -> output
            def make_p2_epilogue(noiseT, outT, pools):
                sb3 = pools

                def epi(sup_i, zp):
                    cols = slice(sup_i * sup, (sup_i + 1) * sup)
                    nn = sb3.tile([fo, sup], F32, tag="nn")
                    nc.sync.dma_start(nn[:], noiseT[:, cols])
                    ex = sb3.tile([fo, sup], F32, tag="ex")
                    nc.scalar.activation(ex[:], zp[fo:2 * fo, :], AF.Exp)
                    pr = sb3.tile([fo, sup], F32, tag="pr")
                    nc.vector.tensor_tensor(out=pr[:], in0=ex[:], in1=nn[:],
                                            op=OP.mult)
                    ob = sb3.tile([fo, sup], F32, tag="ob")
                    nc.vector.tensor_tensor(out=ob[:], in0=pr[:],
                                            in1=zp[0:fo, :], op=OP.add)
                    nc.sync.dma_start(outT[:, cols], ob[:])
                return epi

            with tc.tile_pool(name="meta", bufs=4) as sbm, \
                 tc.tile_pool(name="gath", bufs=3) as sbg, \
                 tc.tile_pool(name="smat", bufs=8) as sbs, \
                 tc.tile_pool(name="zps", bufs=2, space="PSUM") as psp, \
                 tc.tile_pool(name="epi1", bufs=3) as sb2, \
                 tc.tile_pool(name="gps", bufs=4, space="PSUM") as ps2, \
                 tc.tile_pool(name="epi2", bufs=3) as sb3:
                pools = (sbm, sbg, sbs, psp)
                epi_s = make_p1_epilogue(w2s_sb, avw_sb, gs_own, (sb2, ps2))
                epi_t = make_p1_epilogue(w2t_sb, bvw_sb, gt_own, (sb2, ps2))
                for i in range(nsup):   # pass1: fwd on h1s' / bwd on h1t'
                    sparse_pass(i, src_d=idxF, colsc_d=colscF, rs_d=rsF_sb[:],
                                table=h1s_p, fmsg=f1, bias_sb=b1s_sb,
                                zparts=f1, pools=pools, epilogue=epi_s,
                                mdt=BF16,
                                msg_tap=(dbg_msgs if (debug_taps and i == 0)
                                         else None))
                    sparse_pass(i, src_d=idxB, colsc_d=colscB, rs_d=rsB_sb[:],
                                table=h1t_p, fmsg=f1, bias_sb=b1t_sb,
                                zparts=f1, pools=pools, epilogue=epi_t,
                                mdt=BF16)

                if not skip_collectives:
                    nc.gpsimd.collective_compute(
                        "AllGather", OP.bypass, replica_groups=groups,
                        ins=[gs_own[:, :].opt()], outs=[gs_p[:, :].opt()])
                    nc.gpsimd.collective_compute(
                        "AllGather", OP.bypass, replica_groups=groups,
                        ins=[gt_own[:, :].opt()], outs=[gt_p[:, :].opt()])
                if debug_taps:
                    nc.gpsimd.dma_start(dbg_h1s[:, :], h1s_p[:, :])
                    nc.gpsimd.dma_start(dbg_gs[:, :], gs_p[:, :])

                epo_s = make_p2_epilogue(nsT, souT, sb3)
                epo_t = make_p2_epilogue(ntT, touT, sb3)
                for i in range(nsup):   # pass2: bwd on gs' / fwd on gt'
                    sparse_pass(i, src_d=idxB, colsc_d=colscB, rs_d=rsB_sb[:],
                                table=gs_p, fmsg=f2, bias_sb=b2s_sb,
                                zparts=f2, pools=pools, epilogue=epo_s)
                    sparse_pass(i, src_d=idxF, colsc_d=colscF, rs_d=rsF_sb[:],
                                table=gt_p, fmsg=f2, bias_sb=b2t_sb,
                                zparts=f2, pools=pools, epilogue=epo_t)

    nc.compile()
    return nc


# ----------------------------------------------------------------- execution

class Runner:
    """Mirror of bass2jax.run_bass_via_pjrt's multi-core path, but keeps the
    jitted executable + device-resident inputs so repeated calls can be timed
    (this environment has no NTFF profiling hook)."""

    def __init__(self, nc, in_maps, n_cores):
        import jax
        from jax.sharding import Mesh, PartitionSpec, NamedSharding
        from jax.experimental.shard_map import shard_map
        from concourse import bass2jax
        import concourse.mybir as mb

        bass2jax.install_neuronx_cc_hook()
        assert nc.dbg_addr is None or not nc.dbg_callbacks
        if nc.dbg_addr is not None:
            in_maps = [{**m, nc.dbg_addr.name: np.zeros((1, 2), np.uint32)}
                       for m in in_maps]
        partition_name = (nc.partition_id_tensor.name
                          if nc.partition_id_tensor else None)
        in_names, out_names, out_avals, zero_outs = [], [], [], []
        for alloc in nc.m.functions[0].allocations:
            if not isinstance(alloc, mb.MemoryLocationSet):
                continue
            name = alloc.memorylocations[0].name
            if alloc.kind == "ExternalInput":
                if name != partition_name:
                    in_names.append(name)
            elif alloc.kind == "ExternalOutput":
                shape = tuple(alloc.tensor_shape)
                dtype = mb.dt.np(alloc.dtype)
                out_names.append(name)
                out_avals.append(jax.core.ShapedArray(shape, dtype))
                zero_outs.append(np.zeros(shape, dtype))
        n_params = len(in_names)
        all_in_names = list(in_names) + list(out_names)
        if partition_name is not None:
            all_in_names.append(partition_name)

        def _body(*args):
            operands = list(args)
            if partition_name is not None:
                operands.append(bass2jax.partition_id_tensor())
            outs = bass2jax._bass_exec_p.bind(
                *operands, out_avals=tuple(out_avals),
                in_names=tuple(all_in_names), out_names=tuple(out_names),
                lowering_input_output_aliases=(),
                sim_require_finite=True, sim_require_nnan=True, nc=nc)
            return tuple(outs)

        devices = jax.devices()[:n_cores]
        mesh = Mesh(np.asarray(devices), ("core",))
        n_outs = len(out_names)
        donate = tuple(range(n_params, n_params + n_outs))
        self._sharded = jax.jit(
            shard_map(_body, mesh=mesh,
                      in_specs=(PartitionSpec("core"),) * (n_params + n_outs),
                      out_specs=(PartitionSpec("core"),) * n_outs,
                      check_rep=False),
            donate_argnums=donate, keep_unused=True)
        sharding = NamedSharding(mesh, PartitionSpec("core"))
        concat_in = [
            np.concatenate([np.asarray(in_maps[c][nm]) for c in range(n_cores)],
                           axis=0)
            for nm in in_names]
        self._in_dev = [jax.device_put(x, sharding) for x in concat_in]
        self._zeros = [np.zeros((n_cores * z.shape[0], *z.shape[1:]), z.dtype)
                       for z in zero_outs]
        self._sharding = sharding
        self._jax = jax
        self.n_cores = n_cores
        self.out_names = out_names
        self.out_avals = out_avals

    def __call__(self):
        jax = self._jax
        zs = [jax.device_put(z, self._sharding) for z in self._zeros]
        for z in zs:
            z.block_until_ready()
        import time
        t0 = time.perf_counter()
        outs = self._sharded(*self._in_dev, *zs)
        for o in outs:
            o.block_until_ready()
        dt = time.perf_counter() - t0
        results = [
            {name: np.asarray(outs[i]).reshape(self.n_cores,
                                               *self.out_avals[i].shape)[c]
             for i, name in enumerate(self.out_names)}
            for c in range(self.n_cores)]
        return results, dt


_PROGRAM_CACHE = {}


def _get_program(cfg):
    key = tuple(sorted(cfg.items()))
    if key not in _PROGRAM_CACHE:
        _PROGRAM_CACHE[key] = build_program(cfg)
    return _PROGRAM_CACHE[key]


def assemble_outputs(results, cfg):
    souT = np.concatenate([r["souT"] for r in results], axis=1)
    touT = np.concatenate([r["touT"] for r in results], axis=1)
    n = cfg["n"]
    return (np.ascontiguousarray(souT.T[:n]), np.ascontiguousarray(touT.T[:n]))


def run(inputs, timing_iters=0):
    """Run on 8 cores. timing_iters>0: re-invoke via Runner, report wall times;
    default path goes through bass_utils.run_bass_kernel_spmd."""
    in_maps, cfg = prepare_inputs(inputs)
    nc = _get_program(cfg)
    if timing_iters:
        runner = Runner(nc, in_maps, cfg["ncores"])
        results, dt0 = runner()
        times = [dt0]
        for _ in range(timing_iters):
            results, dt = runner()
            times.append(dt)
        return assemble_outputs(results, cfg), times
    res = bass_utils.run_bass_kernel_spmd(
        nc, in_maps, core_ids=list(range(cfg["ncores"])))
    return assemble_outputs(res.results, cfg), [0.0]


def kernel(**inputs):
    out, _ = run(inputs)
    return out



# revision 15
# speedup vs baseline: 1.1855x; 1.1855x over previous
"""Trainium2 Bass kernel for nn_DirectedEdgeEncoder (directed-GCN edge encoder).

Self-contained: hardcodes problem shapes (N=50000, E=800000, IN=128, HID=64,
OUT=32, 8 cores) and builds/runs an SPMD Bass program on NeuronCores 0-7.

Math (reference refactored):
  With self-loops appended, per-edge norm = in_deg[dst]^-.5 * out_deg[src]^-.5
  is IDENTICAL for the forward and flipped conv directions (alpha=beta=0.5).
  Let A[t,s] = sum of norms of edges s->t (incl. self loops). Then
    s_mu||s_logstd = A.T @ ( relu(A @ (s@W1s.T + b1s)) @ W2s.T + b2s )
    t_mu||t_logstd = A   @ ( relu(A.T @ (t@W1t.T + b1t)) @ W2t.T + b2t )
  where W1s = [sm1_W; sl1_W] (128x128), W2s = blockdiag(sm2_W, sl2_W) (64x128).
  outputs: s_out = s_mu + noise_s*exp(s_logstd)/5  (same for t).

  norm factorizes: norm_e = a[dst_e]*b[src_e], a=in_deg^-.5, b=out_deg^-.5.
  The gather-side factor is folded into the stored node tables
  (h1s' = b*h1s for forward gathers, h1t' = a*h1t for backward gathers, etc);
  the scatter-side factor rides in the per-edge selection matrices.
  Biases commute through the conv as  A @ (h + 1 b^T) = A@h + rowsum(A) (x) b,
  implemented as a rank-1 (K=1) matmul seeding the PSUM accumulation.

Device mapping per core (nodes sharded 6400/core, NPAD=51200):
  dense1 (own nodes) -> AllGather h1' -> sparse pass1 (fwd for s, bwd for t):
  per 256-node supertile, one indirect-DMA gather of all edge messages,
  per 128-edge block one DVE tensor_scalar builds S[e,n]=sc_e*(iota==col_e),
  PE accumulates z1T[:,n] += msgs.T @ S in PSUM (float32r, moving dim 256);
  relu (ACT) -> dense2 -> AllGather g' -> sparse pass2 (bwd for s, fwd for t)
  -> combine with noise (ACT exp + DVE) -> transposed outputs [32, 6400].
"""

import math
import numpy as np

import concourse.bass as bass
import concourse.mybir as mybir
import concourse.tile as tile
from concourse import bacc
from concourse import bass_utils

F32 = mybir.dt.float32
F32R = mybir.dt.float32r
I32 = mybir.dt.int32
I16 = mybir.dt.int16
BF16 = mybir.dt.bfloat16
AF = mybir.ActivationFunctionType
OP = mybir.AluOpType

# problem constants
N, E, IN, HID, OUT = 50000, 800000, 128, 64, 32
F1 = 2 * HID   # 128, concat hidden width
F2 = 2 * OUT   # 64, concat output width
LN5INV = float(np.log(np.float32(1.0) / np.float32(5.0)))

# sharding constants
C = 8
SUP = 256
NLOC = 6400          # nodes per core; 25 supertiles
NPAD = C * NLOC      # 51200
NSUP = NLOC // SUP   # 25
TIL = 128


# ----------------------------------------------------------------- host prep

def _pad_rows(x, rows):
    out = np.zeros((rows, x.shape[1]), np.float32)
    out[: x.shape[0]] = x
    return out


def _build_direction(scatter, gather, sc_vals, npad, nloc, sup, ncores, nsup):
    """Sort edges by (scatter supertile, gather-table half); lay out per
    (core, supertile, half) as ranks -> slot (partition rank%128, block
    rank//128, second half offset by KA blocks).  Returns per-half block
    counts [KA, KB] and per-core (gatheridx, col, sc) slot arrays.
    gatheridx values are LOCAL to their table half (for int16 dma_gather)."""
    half_rows = npad // 2
    gsup = scatter // sup                      # global supertile id
    half = gather // half_rows                 # which gather-table half
    key = gsup * 2 + half
    order = np.argsort(key, kind="stable")
    key_s = key[order]
    nkeys = (npad // sup) * 2
    cnt = np.bincount(key_s, minlength=nkeys)
    ka = int(math.ceil(cnt[0::2].max() / 128.0))
    kb = int(math.ceil(cnt[1::2].max() / 128.0))
    starts = np.zeros(nkeys + 1, np.int64)
    np.cumsum(cnt, out=starts[1:])
    rank = np.arange(len(order)) - starts[key_s]
    p = (rank % 128).astype(np.int64)
    blk = (rank // 128).astype(np.int64) + (key_s % 2) * ka
    sup_per_core = nloc // sup
    gsup_s = key_s // 2
    core = gsup_s // sup_per_core
    supl = gsup_s % sup_per_core

    kblk = ka + kb
    src = np.zeros((ncores, nsup, 128, kblk), np.int32)     # pad -> row 0
    col = np.full((ncores, nsup, 128, kblk), -1.0, np.float32)
    sc = np.zeros((ncores, nsup, 128, kblk), np.float32)
    src[core, supl, p, blk] = (gather[order] % half_rows).astype(np.int32)
    col[core, supl, p, blk] = (scatter[order] - gsup_s * sup).astype(np.float32)
    sc[core, supl, p, blk] = sc_vals[order].astype(np.float32)
    return [ka, kb], src, col, sc


def _repad_halves(src, col, sc, kh_from, kh_to):
    """Re-pad per-half block counts [ka,kb] -> uniform [ka',kb']."""
    if kh_from == kh_to:
        return src, col, sc
    outs = []
    for arr, fill in ((src, 0), (col, -1.0), (sc, 0.0)):
        out = np.full(arr.shape[:-1] + (sum(kh_to),), fill, arr.dtype)
        out[..., :kh_from[0]] = arr[..., :kh_from[0]]
        out[..., kh_to[0]:kh_to[0] + kh_from[1]] = arr[..., kh_from[0]:]
        outs.append(out)
    return outs


def _wrap_idx16(src, kh):
    """Slot-array gather indices [C, NSUP, 128, KA+KB] (int32, half-local)
    -> dma_gather idx tiles [C, NSUP, 128, (KA+KB)*8] int16: per half,
    index i at [16-wrap partition i%16, col i//16], replicated x8 down."""
    ncores, nsup = src.shape[0], src.shape[1]
    ka, kb = kh
    out = np.zeros((ncores, nsup, 128, (ka + kb) * 8), np.int16)
    for h, (b0, nb) in enumerate(((0, ka), (ka, kb))):
        if nb == 0:
            continue
        # slot rank i (within half) = block*128 + partition
        sl = src[..., b0:b0 + nb]                       # [C,S,128,nb]
        flat = sl.transpose(0, 1, 3, 2).reshape(ncores, nsup, nb * 128)
        w = flat.reshape(ncores, nsup, nb * 8, 16).astype(np.int16)
        w = w.transpose(0, 1, 3, 2)                     # [C,S,16,nb*8]
        out[..., :16, b0 * 8:(b0 + nb) * 8] = w
        for g in range(1, 8):
            out[..., g * 16:(g + 1) * 16, b0 * 8:(b0 + nb) * 8] = w
    return out


def prepare_inputs(inputs, ncores=C, nloc=NLOC, sup=SUP, n=N):
    """Shard + preprocess full inputs into per-core in_maps. Pure numpy."""
    npad = ncores * nloc
    nsup = nloc // sup
    f32 = np.float32

    s = np.asarray(inputs["s"], f32)
    t = np.asarray(inputs["t"], f32)
    ei = np.asarray(inputs["edge_index"], np.int64)
    noise_s = np.asarray(inputs["noise_s"], f32)
    noise_t = np.asarray(inputs["noise_t"], f32)

    W1s = np.vstack([inputs["sm1_W"], inputs["sl1_W"]]).astype(f32)   # [F1, IN]
    W1t = np.vstack([inputs["tm1_W"], inputs["tl1_W"]]).astype(f32)
    b1s = np.concatenate([inputs["sm1_b"], inputs["sl1_b"]]).astype(f32)
    b1t = np.concatenate([inputs["tm1_b"], inputs["tl1_b"]]).astype(f32)
    f2, f1 = 2 * inputs["sm2_W"].shape[0], 2 * inputs["sm1_W"].shape[0]
    W2s = np.zeros((f2, f1), f32)
    W2s[: f2 // 2, : f1 // 2] = inputs["sm2_W"]
    W2s[f2 // 2:, f1 // 2:] = inputs["sl2_W"]
    W2t = np.zeros((f2, f1), f32)
    W2t[: f2 // 2, : f1 // 2] = inputs["tm2_W"]
    W2t[f2 // 2:, f1 // 2:] = inputs["tl2_W"]
    b2s = np.concatenate([inputs["sm2_b"], inputs["sl2_b"]]).astype(f32)
    b2t = np.concatenate([inputs["tm2_b"], inputs["tl2_b"]]).astype(f32)

    loops = np.arange(n, dtype=np.int64)
    src = np.concatenate([ei[0], loops])
    dst = np.concatenate([ei[1], loops])
    deg_in = np.bincount(dst, minlength=npad).astype(f32)
    deg_out = np.bincount(src, minlength=npad).astype(f32)
    a = (np.maximum(deg_in, 1.0) ** -0.5).astype(f32)    # scatter-side fwd
    b = (np.maximum(deg_out, 1.0) ** -0.5).astype(f32)
    rs_fwd = a * np.bincount(dst, weights=b[src].astype(np.float64),
                             minlength=npad).astype(f32)
    rs_bwd = b * np.bincount(src, weights=a[dst].astype(np.float64),
                             minlength=npad).astype(f32)

    # forward direction (matrix A): scatter by dst, gather at src, factor a[dst]
    khf, srcF, colF, scF = _build_direction(dst, src, a[dst], npad, nloc, sup,
                                            ncores, nsup)
    # backward direction (A.T): scatter by src, gather at dst, factor b[src]
    khb, srcB, colB, scB = _build_direction(src, dst, b[src], npad, nloc, sup,
                                            ncores, nsup)
    kh = [max(khf[0], khb[0]), max(khf[1], khb[1])]
    srcF, colF, scF = _repad_halves(srcF, colF, scF, khf, kh)
    srcB, colB, scB = _repad_halves(srcB, colB, scB, khb, kh)
    kblk = kh[0] + kh[1]
    idxF = _wrap_idx16(srcF, kh)
    idxB = _wrap_idx16(srcB, kh)

    def colsc(col, sc):  # pack [.., 128, 4K]: col, sc, -col, -sc
        return np.concatenate([col, sc, -col, -sc], axis=-1)

    sT = _pad_rows(s, npad).T.copy()           # [IN, npad]
    tT = _pad_rows(t, npad).T.copy()
    # /DIVIDER folded into the noise here (out = mu + (noise/5) * exp(logstd))
    nsT = (_pad_rows(noise_s, npad).T / np.float32(5.0)).astype(f32)  # [OUT, npad]
    ntT = (_pad_rows(noise_t, npad).T / np.float32(5.0)).astype(f32)

    import ml_dtypes
    bf16 = ml_dtypes.bfloat16
    iota = np.tile(np.arange(sup, dtype=f32), (128, 1)).astype(bf16)
    in_maps = []
    for c in range(ncores):
        r = slice(c * nloc, (c + 1) * nloc)
        in_maps.append({
            "xsT": np.ascontiguousarray(sT[:, r]).astype(bf16),
            "xtT": np.ascontiguousarray(tT[:, r]).astype(bf16),
            "w1sT": np.ascontiguousarray(W1s.T).astype(bf16),
            "w1tT": np.ascontiguousarray(W1t.T).astype(bf16),
            "w2sT": np.ascontiguousarray(W2s.T).astype(bf16),
            "w2tT": np.ascontiguousarray(W2t.T).astype(bf16),
            "b1s": b1s[None, :].astype(bf16),
            "b1t": b1t[None, :].astype(bf16),
            "b2s": b2s[None, :].astype(bf16),
            "b2t": b2t[None, :].astype(bf16),
            "avw": a[r].reshape(nloc // 128, 128).T.copy(),
            "bvw": b[r].reshape(nloc // 128, 128).T.copy(),
            "rsF": rs_fwd[None, r].astype(bf16),
            "rsB": rs_bwd[None, r].astype(bf16),
            "iota": iota,
            "idxF": idxF[c], "colscF": colsc(colF[c], scF[c]),
            "idxB": idxB[c], "colscB": colsc(colB[c], scB[c]),
            "srcF": srcF[c], "srcB": srcB[c],   # debug only (unused by NEFF)
            "nsT": np.ascontiguousarray(nsT[:, r]),
            "ntT": np.ascontiguousarray(ntT[:, r]),
        })
    cfg = dict(ncores=ncores, nloc=nloc, npad=npad, sup=sup, nsup=nsup,
               kblk=kblk, ka=kh[0], kb=kh[1], fin=s.shape[1], f1=W1s.shape[0],
               f2=W2s.shape[0], fo=W2s.shape[0] // 2, n=n)
    return in_maps, cfg


# ------------------------------------------------------------ device program

def build_program(cfg, enable_asserts=False, debug_taps=False, repeat=1, skip_collectives=False, ablate=()):
    ncores = cfg["ncores"]
    nloc, npad, sup, nsup = cfg["nloc"], cfg["npad"], cfg["sup"], cfg["nsup"]
    kblk, fin, f1, f2, fo = (cfg["kblk"], cfg["fin"], cfg["f1"], cfg["f2"],
                             cfg["fo"])
    ntil = nloc // TIL

    nswq = 4
    nc = bacc.Bacc("TRN2", target_bir_lowering=False, debug=False,
                   enable_asserts=enable_asserts, num_devices=ncores,
                   dynamic_dma_scratch_size=65536, num_swdge_queues=nswq)
    qctr = [0]

    def din(name, shape, dtype=F32):
        return nc.dram_tensor(name, shape, dtype, kind="ExternalInput")

    xsT = din("xsT", [fin, nloc], BF16)
    xtT = din("xtT", [fin, nloc], BF16)
    w1sT = din("w1sT", [fin, f1], BF16)
    w1tT = din("w1tT", [fin, f1], BF16)
    w2sT = din("w2sT", [f1, f2], BF16)
    w2tT = din("w2tT", [f1, f2], BF16)
    b1s = din("b1s", [1, f1], BF16)
    b1t = din("b1t", [1, f1], BF16)
    b2s = din("b2s", [1, f2], BF16)
    b2t = din("b2t", [1, f2], BF16)
    avw = din("avw", [128, nloc // TIL])
    bvw = din("bvw", [128, nloc // TIL])
    rsF = din("rsF", [1, nloc], BF16)
    rsB = din("rsB", [1, nloc], BF16)
    iota = din("iota", [128, sup], BF16)
    idxF = din("idxF", [nsup, 128, kblk * 8], I16)
    colscF = din("colscF", [nsup, 128, 4 * kblk])
    idxB = din("idxB", [nsup, 128, kblk * 8], I16)
    colscB = din("colscB", [nsup, 128, 4 * kblk])
    nsT = din("nsT", [fo, nloc])
    ntT = din("ntT", [fo, nloc])

    souT = nc.dram_tensor("souT", [fo, nloc], F32, kind="ExternalOutput")
    touT = nc.dram_tensor("touT", [fo, nloc], F32, kind="ExternalOutput")
    dbg_h1s = dbg_gs = dbg_msgs = None
    if debug_taps:
        dbg_h1s = nc.dram_tensor("dbg_h1s", [npad, f1], F32,
                                 kind="ExternalOutput")
        dbg_gs = nc.dram_tensor("dbg_gs", [npad, 2 * f2], F32,
                                kind="ExternalOutput")
        dbg_msgs = nc.dram_tensor("dbg_msgs", [128, kblk * f1], F32,
                                  kind="ExternalOutput")

    cc_space = "Shared" if ncores > 4 else "Local"
    h1s_own = nc.dram_tensor("h1s_own", [nloc, f1], BF16, kind="Internal")
    h1t_own = nc.dram_tensor("h1t_own", [nloc, f1], BF16, kind="Internal")
    h1s_p = nc.dram_tensor("h1s_p", [npad, f1], BF16, kind="Internal",
                           addr_space=cc_space)
    h1t_p = nc.dram_tensor("h1t_p", [npad, f1], BF16, kind="Internal",
                           addr_space=cc_space)
    # combined [gs | gt] table: 256B rows keep dma_gather legal at bf16
    g_own = nc.dram_tensor("g_own", [nloc, 2 * f2], BF16, kind="Internal")
    g_p = nc.dram_tensor("g_p", [npad, 2 * f2], BF16, kind="Internal",
                         addr_space=cc_space)

    groups = [list(range(ncores))]

    with tile.TileContext(nc) as tc:
      if skip_collectives:
          # timing-only variant: gathers must not touch uninitialized HBM
          with tc.tile_pool(name="zinit", bufs=1) as zp:
              zt = zp.tile([128, 3200], BF16, tag="zt")
              nc.gpsimd.memset(zt[:], 0.0)
              for tbl, fdim in ((h1s_p, f1), (h1t_p, f1), (g_p, 2 * f2)):
                  view = tbl[:, :].rearrange("(nn p) f -> p nn f", p=128)
                  zt3 = zt[:].rearrange("p (nn f) -> p nn f", f=fdim)
                  nn_tot, nn_ch = view.shape[1], 3200 // fdim
                  for c0 in range(0, nn_tot, nn_ch):
                      cw = min(nn_ch, nn_tot - c0)
                      nc.gpsimd.dma_start(view[:, c0:c0 + cw, :],
                                          zt3[:, :cw, :])
      for _rep in range(repeat):
        with tc.tile_pool(name="const", bufs=1) as cp:
            w1s_sb = cp.tile([fin, f1], BF16, tag="w1s")
            nc.sync.dma_start(w1s_sb[:], w1sT[:, :])
            w1t_sb = cp.tile([fin, f1], BF16, tag="w1t")
            nc.sync.dma_start(w1t_sb[:], w1tT[:, :])
            w2s_sb = cp.tile([f1, f2], BF16, tag="w2s")
            nc.sync.dma_start(w2s_sb[:], w2sT[:, :])
            w2t_sb = cp.tile([f1, f2], BF16, tag="w2t")
            nc.sync.dma_start(w2t_sb[:], w2tT[:, :])
            b1s_sb = cp.tile([1, f1], BF16, tag="b1s")
            nc.sync.dma_start(b1s_sb[:], b1s[:, :])
            b1t_sb = cp.tile([1, f1], BF16, tag="b1t")
            nc.sync.dma_start(b1t_sb[:], b1t[:, :])
            b2s_sb = cp.tile([1, f2], BF16, tag="b2s")
            nc.sync.dma_start(b2s_sb[:], b2s[:, :])
            b2t_sb = cp.tile([1, f2], BF16, tag="b2t")
            nc.sync.dma_start(b2t_sb[:], b2t[:, :])
            iota_sb = cp.tile([128, sup], BF16, tag="iota")
            nc.sync.dma_start(iota_sb[:], iota[:, :])
            avw_sb = cp.tile([128, nloc // TIL], F32, tag="avw")
            nc.sync.dma_start(avw_sb[:], avw[:, :])
            bvw_sb = cp.tile([128, nloc // TIL], F32, tag="bvw")
            nc.sync.dma_start(bvw_sb[:], bvw[:, :])
            rsF_sb = cp.tile([1, nloc], BF16, tag="rsF")
            nc.sync.dma_start(rsF_sb[:], rsF[:, :])
            rsB_sb = cp.tile([1, nloc], BF16, tag="rsB")
            nc.sync.dma_start(rsB_sb[:], rsB[:, :])

            # ---------------- dense layer 1 on own nodes ----------------
            # s first, AllGather(h1s) issued ASAP so it overlaps dense1-t;
            # AllGather(h1t) then overlaps the pass1-s sparse supertiles.
            with tc.tile_pool(name="d1", bufs=3) as sb, \
                 tc.tile_pool(name="d1ps", bufs=4, space="PSUM") as ps:
                for xT, w1, svw, h1own, h1p in (
                    (xsT, w1s_sb, bvw_sb, h1s_own, h1s_p),  # fwd gathers: b*h
                    (xtT, w1t_sb, avw_sb, h1t_own, h1t_p),  # bwd gathers: a*h
                ):
                    for j in range(ntil):
                        rows = slice(j * TIL, (j + 1) * TIL)
                        xt = sb.tile([fin, TIL], BF16, tag="xt")
                        nc.sync.dma_start(xt[:], xT[:, rows])
                        p = ps.tile([TIL, f1], F32, tag="d1p")
                        nc.tensor.matmul(p[:], lhsT=xt[:],
                                         rhs=w1[:],
                                         start=True, stop=True)
                        h1sb = sb.tile([TIL, f1], BF16, tag="h1sb")
                        nc.scalar.activation(h1sb[:], p[:], AF.Copy,
                                             scale=svw[:, j:j + 1])
                        nc.sync.dma_start(h1own[rows, :], h1sb[:])
                    if not skip_collectives:
                        nc.gpsimd.collective_compute(
                            "AllGather", OP.bypass, replica_groups=groups,
                            ins=[h1own[:, :].opt()], outs=[h1p[:, :].opt()])

            # ---------------- sparse passes ----------------
            halfrows = npad // 2
            kab = [cfg["ka"], cfg["kb"]]

            ACT_EVERY = 4   # build every 4th S on the (idle) scalar engine

            def sparse_pass(sup_i, *, src_d, colsc_d, rs_d, table,
                            bias_sb, zparts, pools, epilogue, foff=0,
                            fuse=None, msg_tap=None):
                fuse = f1 if fuse is None else fuse
                sbm, sbg, sbs, psp = pools
                srct = sbm.tile([128, kblk * 8], I16, tag="src")
                nc.sync.dma_start(srct[:], src_d[sup_i])
                cst = sbm.tile([128, 4 * kblk], F32, tag="colsc")
                nc.sync.dma_start(cst[:], colsc_d[sup_i])
                rst = rs_d[0:1, sup_i * sup:(sup_i + 1) * sup]
                msgs = sbg.tile([128, kblk * f1], BF16, tag="msgs")
                msgs3 = msgs[:].rearrange("p (k f) -> p k f", k=kblk)
                GCH = 8    # dma_gather HW limit: 1024 idxs (64/16-lane) per op
                if "gather" not in ablate:
                    for h, (b0, nb) in enumerate(((0, kab[0]), (kab[0], kab[1]))):
                        for c0 in range(0, nb, GCH):
                            cn = min(GCH, nb - c0)
                            qctr[0] += 1
                            nc.gpsimd.dma_gather(
                                out_ap=msgs3[:, b0 + c0:b0 + c0 + cn, :],
                                in_ap=table[h * halfrows:(h + 1) * halfrows, :],
                                idxs_ap=srct[:, (b0 + c0) * 8:(b0 + c0 + cn) * 8],
                                num_idxs=cn * 128, num_idxs_reg=cn * 128,
                                elem_size=f1, queue_num=qctr[0] % nswq)
                if msg_tap is not None:
                    nc.gpsimd.dma_start(msg_tap[:, :], msgs[:])
                if "epi" in ablate:
                    return
                zp = psp.tile([zparts, sup], F32, tag=f"z{zparts}")
                nc.tensor.matmul(zp[:], lhsT=bias_sb[:], rhs=rst,
                                 start=True, stop=("mm" in ablate))
                for k in range(kblk):
                    S = sbs.tile([128, sup], BF16, tag="S")
                    if "sbuild" not in ablate:
                        if k % ACT_EVERY == ACT_EVERY - 1:
                            # one-hot via ACT: relu(sc - sc*|iota - col|)
                            tmp = sbs.tile([128, sup], BF16, tag="Stmp")
                            nc.scalar.activation(
                                tmp[:], iota_sb[:], AF.Abs,
                                bias=cst[:, 2 * kblk + k:2 * kblk + k + 1])
                            nc.scalar.activation(
                                S[:], tmp[:], AF.Relu,
                                scale=cst[:, 3 * kblk + k:3 * kblk + k + 1],
                                bias=cst[:, kblk + k:kblk + k + 1])
                        else:
                            nc.vector.tensor_scalar(
                                out=S[:], in0=iota_sb[:],
                                scalar1=cst[:, k:k + 1],
                                scalar2=cst[:, kblk + k:kblk + k + 1],
                                op0=OP.is_equal, op1=OP.mult)
                    if "mm" not in ablate:
                        nc.tensor.matmul(
                            zp[:],
                            lhsT=msgs[:, k * f1 + foff:k * f1 + foff + fuse],
                            rhs=S[:],
                            start=False, stop=(k == kblk - 1))
                epilogue(sup_i, zp)

            # pass 1 -> r1 -> dense2 -> g_own half (combined [gs|gt] table)
            def make_p1_epilogue(w2_sb, svw, goff, pools):
                sb2, ps2 = pools

                def epi(sup_i, zp):
                    r1 = sb2.tile([f1, sup], BF16, tag="r1")
                    nc.scalar.activation(r1[:], zp[:], AF.Relu)
                    for h in range(sup // TIL):
                        j = sup_i * (sup // TIL) + h
                        rows = slice(j * TIL, (j + 1) * TIL)
                        gp = ps2.tile([TIL, f2], F32, tag="gp")
                        nc.tensor.matmul(
                            gp[:],
                            lhsT=r1[:, h * TIL:(h + 1) * TIL],
                            rhs=w2_sb[:], start=True, stop=True)
                        gsb = sb2.tile([TIL, f2], BF16, tag="gsb")
                        nc.scalar.activation(gsb[:], gp[:], AF.Copy,
                                             scale=svw[:, j:j + 1])
                        nc.sync.dma_start(g_own[rows, goff:goff + f2], gsb[:])
                return epi

            # pass 2 -> combine with noise -> output
            def make_p2_epilogue(noiseT, outT, pools):
                sb3 = pools

                def epi(sup_i, zp):
                    cols = slice(sup_i * sup, (sup_i + 1) * sup)
                    nn = sb3.tile([fo, sup], F32, tag="nn")
                    nc.sync.dma_start(nn[:], noiseT[:, cols])
                    ex = sb3.tile([fo, sup], F32, tag="ex")
                    nc.scalar.activation(ex[:], zp[fo:2 * fo, :], AF.Exp)
                    pr = sb3.tile([fo, sup], F32, tag="pr")
                    nc.vector.tensor_tensor(out=pr[:], in0=ex[:], in1=nn[:],
                                            op=OP.mult)
                    ob = sb3.tile([fo, sup], F32, tag="ob")
                    nc.vector.tensor_tensor(out=ob[:], in0=pr[:],
                                            in1=zp[0:fo, :], op=OP.add)
                    nc.sync.dma_start(outT[:, cols], ob[:])
                return epi

            with tc.tile_pool(name="meta", bufs=4) as sbm, \
                 tc.tile_pool(name="gath", bufs=3) as sbg, \
                 tc.tile_pool(name="smat", bufs=8) as sbs, \
                 tc.tile_pool(name="zps", bufs=2, space="PSUM") as psp, \
                 tc.tile_pool(name="epi1", bufs=3) as sb2, \
                 tc.tile_pool(name="gps", bufs=4, space="PSUM") as ps2, \
                 tc.tile_pool(name="epi2", bufs=3) as sb3:
                pools = (sbm, sbg, sbs, psp)
                epi_s = make_p1_epilogue(w2s_sb, avw_sb, 0, (sb2, ps2))
                epi_t = make_p1_epilogue(w2t_sb, bvw_sb, f2, (sb2, ps2))
                for i in range(nsup):   # pass1-s: fwd on h1s'
                    sparse_pass(i, src_d=idxF, colsc_d=colscF, rs_d=rsF_sb[:],
                                table=h1s_p, bias_sb=b1s_sb,
                                zparts=f1, pools=pools, epilogue=epi_s,
                                msg_tap=(dbg_msgs if (debug_taps and i == 0)
                                         else None))
                for i in range(nsup):   # pass1-t: bwd on h1t'
                    sparse_pass(i, src_d=idxB, colsc_d=colscB, rs_d=rsB_sb[:],
                                table=h1t_p, bias_sb=b1t_sb,
                                zparts=f1, pools=pools, epilogue=epi_t)

                if not skip_collectives:
                    nc.gpsimd.collective_compute(
                        "AllGather", OP.bypass, replica_groups=groups,
                        ins=[g_own[:, :].opt()], outs=[g_p[:, :].opt()])
                if debug_taps:
                    nc.gpsimd.dma_start(dbg_h1s[:, :], h1s_p[:, :])
                    nc.gpsimd.dma_start(dbg_gs[:, :], g_p[:, :])

                epo_s = make_p2_epilogue(nsT, souT, sb3)
                epo_t = make_p2_epilogue(ntT, touT, sb3)
                for i in range(nsup):   # pass2-s: bwd on gs half of g'
                    sparse_pass(i, src_d=idxB, colsc_d=colscB, rs_d=rsB_sb[:],
                                table=g_p, bias_sb=b2s_sb, foff=0, fuse=f2,
                                zparts=f2, pools=pools, epilogue=epo_s)
                for i in range(nsup):   # pass2-t: fwd on gt half of g'
                    sparse_pass(i, src_d=idxF, colsc_d=colscF, rs_d=rsF_sb[:],
                                table=g_p, bias_sb=b2t_sb, foff=f2, fuse=f2,
                                zparts=f2, pools=pools, epilogue=epo_t)

    nc.compile()
    return nc


# ----------------------------------------------------------------- execution

class Runner:
    """Mirror of bass2jax.run_bass_via_pjrt's multi-core path, but keeps the
    jitted executable + device-resident inputs so repeated calls can be timed
    (this environment has no NTFF profiling hook)."""

    def __init__(self, nc, in_maps, n_cores):
        import jax
        from jax.sharding import Mesh, PartitionSpec, NamedSharding
        from jax.experimental.shard_map import shard_map
        from concourse import bass2jax
        import concourse.mybir as mb

        bass2jax.install_neuronx_cc_hook()
        assert nc.dbg_addr is None or not nc.dbg_callbacks
        if nc.dbg_addr is not None:
            in_maps = [{**m, nc.dbg_addr.name: np.zeros((1, 2), np.uint32)}
                       for m in in_maps]
        partition_name = (nc.partition_id_tensor.name
                          if nc.partition_id_tensor else None)
        in_names, out_names, out_avals, zero_outs = [], [], [], []
        for alloc in nc.m.functions[0].allocations:
            if not isinstance(alloc, mb.MemoryLocationSet):
                continue
            name = alloc.memorylocations[0].name
            if alloc.kind == "ExternalInput":
                if name != partition_name:
                    in_names.append(name)
            elif alloc.kind == "ExternalOutput":
                shape = tuple(alloc.tensor_shape)
                dtype = mb.dt.np(alloc.dtype)
                out_names.append(name)
                out_avals.append(jax.core.ShapedArray(shape, dtype))
                zero_outs.append(np.zeros(shape, dtype))
        n_params = len(in_names)
        all_in_names = list(in_names) + list(out_names)
        if partition_name is not None:
            all_in_names.append(partition_name)

        def _body(*args):
            operands = list(args)
            if partition_name is not None:
                operands.append(bass2jax.partition_id_tensor())
            outs = bass2jax._bass_exec_p.bind(
                *operands, out_avals=tuple(out_avals),
                in_names=tuple(all_in_names), out_names=tuple(out_names),
                lowering_input_output_aliases=(),
                sim_require_finite=True, sim_require_nnan=True, nc=nc)
            return tuple(outs)

        devices = jax.devices()[:n_cores]
        mesh = Mesh(np.asarray(devices), ("core",))
        n_outs = len(out_names)
        donate = tuple(range(n_params, n_params + n_outs))
        self._sharded = jax.jit(
            shard_map(_body, mesh=mesh,
                      in_specs=(PartitionSpec("core"),) * (n_params + n_outs),
                      out_specs=(PartitionSpec("core"),) * n_outs,
                      check_rep=False),
            donate_argnums=donate, keep_unused=True)
        sharding = NamedSharding(mesh, PartitionSpec("core"))
        concat_in = [
            np.concatenate([np.asarray(in_maps[c][nm]) for c in range(n_cores)],
                           axis=0)
            for nm in in_names]
        self._in_dev = [jax.device_put(x, sharding) for x in concat_in]
        self._zeros = [np.zeros((n_cores * z.shape[0], *z.shape[1:]), z.dtype)
                       for z in zero_outs]
        self._sharding = sharding
        self._jax = jax
        self.n_cores = n_cores
        self.out_names = out_names
        self.out_avals = out_avals

    def __call__(self):
        jax = self._jax
        zs = [jax.device_put(z, self._sharding) for z in self._zeros]
        for z in zs:
            z.block_until_ready()
        import time
        t0 = time.perf_counter()
        outs = self._sharded(*self._in_dev, *zs)
        for o in outs:
            o.block_until_ready()
        dt = time.perf_counter() - t0
        results = [
            {name: np.asarray(outs[i]).reshape(self.n_cores,
                                               *self.out_avals[i].shape)[c]
             for i, name in enumerate(self.out_names)}
            for c in range(self.n_cores)]
        return results, dt


_PROGRAM_CACHE = {}


def _get_program(cfg):
    key = tuple(sorted(cfg.items()))
    if key not in _PROGRAM_CACHE:
        _PROGRAM_CACHE[key] = build_program(cfg)
    return _PROGRAM_CACHE[key]


def assemble_outputs(results, cfg):
    souT = np.concatenate([r["souT"] for r in results], axis=1)
    touT = np.concatenate([r["touT"] for r in results], axis=1)
    n = cfg["n"]
    return (np.ascontiguousarray(souT.T[:n]), np.ascontiguousarray(touT.T[:n]))


def run(inputs, timing_iters=0):
    """Run on 8 cores. timing_iters>0: re-invoke via Runner, report wall times;
    default path goes through bass_utils.run_bass_kernel_spmd."""
    in_maps, cfg = prepare_inputs(inputs)
    nc = _get_program(cfg)
    if timing_iters:
        runner = Runner(nc, in_maps, cfg["ncores"])
        results, dt0 = runner()
        times = [dt0]
        for _ in range(timing_iters):
            results, dt = runner()
            times.append(dt)
        return assemble_outputs(results, cfg), times
    res = bass_utils.run_bass_kernel_spmd(
        nc, in_maps, core_ids=list(range(cfg["ncores"])))
    return assemble_outputs(res.results, cfg), [0.0]


def kernel(**inputs):
    out, _ = run(inputs)
    return out

